# revision 1
# baseline (speedup 1.0000x reference)
"""DiscreteBipartiteFlow forward on 8 Trainium2 NeuronCores.

Math (forward pass only):
  masked = mask * inputs                      (mask = 1 at odd l, 0 at even l)
  h   = relu(masked.reshape(B, L*V) @ W1 + b1)
  net = (h @ W2 + b2).reshape(B, L, 2V)
  loc, scale = argmax one-hots of net[..., :V], net[..., V:]
  out[odd l]  = inputs
  out[even l] = onehot((inv(scale) * ((tok - loc) mod V)) mod V), or 0 if scale==0

st_one_hot_argmax's forward value is exactly the hard one-hot (soft terms
cancel), so the post-MLP flow is pure index arithmetic mod 23.

Sharding (8 cores):
  mm1: tensor-parallel over hidden. Core k computes hT[512k:512k+512, :] from
       the odd-position one-hot rows only (mask zeroes even rows; 2944 of 5888
       W1 rows ever contribute).
  all-gather: h, split into bf16 hi+lo, gathered in 4 chunks (one per local
       128-row tile) so communication pipelines under mm2; a zero-dep dummy
       collective at t=0 absorbs cross-core start skew + comm setup.
  mm2: tensor-parallel over output columns. Core k owns positions
       l in [32k, 32k+32) = 1472 columns of W2. The contraction loop is
       grouped by gather chunk (j % 4) so each chunk's matmuls start as soon
       as its gather lands.
  epilogue: per-core argmax + modular flow for its 16 even positions;
       host interleaves position slices and passes odd positions through.

Precision: matmuls run as bf16 hi/lo split passes (x one-hot is exact in
bf16, so mm1 = 2 passes over W1{hi,lo}; mm2 = 3 passes hh+hl+lh) with fp32
PSUM accumulation -> ~2^-18 operand error, fp32-grade argmax fidelity, at
1 cycle/row TensorE throughput (vs 4 cycles/row for native fp32).
"""

import numpy as np
import ml_dtypes

B, L, V = 512, 256, 23
H = 4096
NCORES = 8
HS = H // NCORES          # 512  hidden shard
HM = HS // 128            # 4    local hidden tiles
PS = L // NCORES          # 32   positions per core
EP = PS // 2              # 16   even positions per core
CW = PS * 2 * V           # 1472 net columns per core (incl. unused odd)
CE = EP * 2 * V           # 736  even-position net columns (the used ones)
NCH = 2                   # column chunks for mm2
CC = CE // NCH            # 368  columns per chunk (8 even positions)
KT1 = (L // 2) * V // 128  # 23 contraction tiles for mm1
KT2 = H // 128            # 32 contraction tiles for mm2
MT = B // 128             # 4 batch tiles

BIG = 64.0
MAGIC = 12582912.0        # 1.5 * 2^23: float32 round-to-int domain
BF16 = ml_dtypes.bfloat16

_cache = {}


def _build():
    import concourse.mybir as mybir
    import concourse.tile as tile
    from concourse import bacc

    fp32 = mybir.dt.float32
    bf16 = mybir.dt.bfloat16
    Alu = mybir.AluOpType
    Act = mybir.ActivationFunctionType

    nc = bacc.Bacc("TRN2", target_bir_lowering=False, debug=False,
                   num_devices=NCORES)

    # ---- per-core inputs ----
    xt = nc.dram_tensor("xt", [KT1, 128, B], bf16, kind="ExternalInput")
    w1h = nc.dram_tensor("w1h", [KT1, 128, HS], bf16, kind="ExternalInput")
    w1l = nc.dram_tensor("w1l", [KT1, 128, HS], bf16, kind="ExternalInput")
    b1s = nc.dram_tensor("b1s", [HM, 128], fp32, kind="ExternalInput")
    # W2 pre-tiled on host: per j-tile one contiguous [128, 8*CC] block
    # holding (hi,lo) x 4 column chunks -> one big DMA per contraction tile
    w2a = nc.dram_tensor("w2a", [KT2, 128, 2 * NCH * CC], bf16,
                         kind="ExternalInput")
    b2r = nc.dram_tensor("b2r", [128, CE], fp32, kind="ExternalInput")
    inpe = nc.dram_tensor("inpe", [MT, 128, EP * V], fp32, kind="ExternalInput")
    oute = nc.dram_tensor("oute", [MT, 128, EP * V], fp32, kind="ExternalOutput")

    # ---- constants (baked into the NEFF) ----
    iota_np = np.arange(V, dtype=np.float32)[None, :].repeat(128, 0)
    c_iota = nc.inline_tensor(np.ascontiguousarray(iota_np), name="c_iota")
    c_bi = nc.inline_tensor(np.ascontiguousarray(BIG - iota_np), name="c_bi")

    with tile.TileContext(nc) as tc:
        with (
            tc.tile_pool(name="persist", bufs=1) as persist,
            tc.tile_pool(name="hwork", bufs=2) as hwork,
            tc.tile_pool(name="w2s", bufs=4) as w2s,
            tc.tile_pool(name="ep", bufs=2) as ep,
            tc.tile_pool(name="small", bufs=2) as small,
            tc.tile_pool(name="ps", bufs=1, space="PSUM") as ps,
            tc.tile_pool(name="dram", bufs=1, space="DRAM") as dram,
        ):
            # ---------- dummy collective: absorbs start skew + comm setup ---
            warm_in = dram.tile([1, 16], fp32, tag="warm_in")
            warm_out = dram.tile([NCORES, 16], fp32, tag="warm_out",
                                 addr_space="Shared")
            nc.gpsimd.collective_compute(
                "AllGather", Alu.bypass,
                replica_groups=[list(range(NCORES))],
                ins=[warm_in.opt()], outs=[warm_out.opt()],
            )

            # ---------- constants to SBUF ----------
            iota_t = persist.tile([128, V], fp32, tag="iota")
            nc.sync.dma_start(iota_t[:], c_iota[:])
            cbi_t = persist.tile([128, V], fp32, tag="cbi")
            nc.sync.dma_start(cbi_t[:], c_bi[:])

            # ---------- load mm1 operands ----------
            xt_t, w1h_t, w1l_t = [], [], []
            for k in range(KT1):
                t = persist.tile([128, B], bf16, tag=f"xt{k}")
                nc.sync.dma_start(t[:], xt[k])
                xt_t.append(t)
                th = persist.tile([128, HS], bf16, tag=f"w1h{k}")
                nc.sync.dma_start(th[:], w1h[k])
                w1h_t.append(th)
                tl = persist.tile([128, HS], bf16, tag=f"w1l{k}")
                nc.sync.dma_start(tl[:], w1l[k])
                w1l_t.append(tl)
            b1_t = []
            for m in range(HM):
                t = persist.tile([128, 1], fp32, tag=f"b1{m}")
                nc.sync.dma_start(t[:], b1s[m].unsqueeze(1))
                b1_t.append(t)

            # ---------- per-chunk collective buffers ----------
            ag_in = [dram.tile([2, 128, B], bf16, tag=f"ag_in{m}",
                               name=f"ag_in{m}") for m in range(HM)]
            ag_out = [dram.tile([NCORES, 2, 128, B], bf16, tag=f"ag_out{m}",
                                name=f"ag_out{m}", addr_space="Shared")
                      for m in range(HM)]

            # ---------- phase 1: mm1 -> local hT tile, relu, split, gather --
            hhi_loc, hlo_loc = [], []
            for m in range(HM):
                acc = ps.tile([128, B], fp32, tag=f"p4_{m}_0", name=f"ps1_{m}")
                for k in range(KT1):
                    nc.tensor.matmul(acc[:], w1h_t[k][:, m * 128:(m + 1) * 128],
                                     xt_t[k][:], start=(k == 0), stop=False)
                for k in range(KT1):
                    nc.tensor.matmul(acc[:], w1l_t[k][:, m * 128:(m + 1) * 128],
                                     xt_t[k][:], start=False, stop=(k == KT1 - 1))
                hf = hwork.tile([128, B], fp32, tag="hf")
                nc.scalar.activation(hf[:], acc[:], Act.Relu, bias=b1_t[m], scale=1.0)
                hhi = hwork.tile([128, B], bf16, tag=f"hhi{m}", bufs=1)
                nc.vector.tensor_copy(hhi[:], hf[:])
                hlo = hwork.tile([128, B], bf16, tag=f"hlo{m}", bufs=1)
                nc.vector.tensor_sub(hlo[:], hf[:], hhi[:])
                hhi_loc.append(hhi)
                hlo_loc.append(hlo)
                nc.sync.dma_start(ag_in[m][0], hhi[:])
                nc.sync.dma_start(ag_in[m][1], hlo[:])
                nc.gpsimd.collective_compute(
                    "AllGather", Alu.bypass,
                    replica_groups=[list(range(NCORES))],
                    ins=[ag_in[m].opt()], outs=[ag_out[m].opt()],
                )

            b2_t = persist.tile([128, CE], fp32, tag="b2")
            nc.sync.dma_start(b2_t[:], b2r[:])
            # token index per batch tile (dep-free, runs during mm1)
            t_tok = []
            for m in range(MT):
                it = ep.tile([128, EP * V], fp32, tag="inpe")
                nc.sync.dma_start(it[:], inpe[m])
                tk = persist.tile([128, EP], fp32, tag=f"tok{m}")
                tmp = ep.tile([128, EP, V], fp32, tag="tokmul")
                nc.vector.tensor_tensor(
                    tmp[:], it[:].rearrange("p (e v) -> p e v", v=V),
                    iota_t[:].unsqueeze(1).broadcast_to([128, EP, V]), Alu.mult)
                nc.vector.tensor_reduce(tk[:], tmp[:], axis=mybir.AxisListType.X,
                                        op=Alu.add)
                t_tok.append(tk)

            # ---------- phase 3: gathered hT to SBUF (per chunk) ----------
            # reuse the xt/w1 tile slots -- same size, dead after mm1
            recycle = ([f"xt{k}" for k in range(KT1)]
                       + [f"w1h{k}" for k in range(KT1)]
                       + [f"w1l{k}" for k in range(KT1)])
            hth_t, htl_t = [None] * KT2, [None] * KT2
            for g in range(HM):
                for s in range(NCORES):
                    j = HM * s + g
                    th = persist.tile([128, B], bf16, tag=recycle[2 * j],
                                      name=f"hth{j}")
                    nc.sync.dma_start(th[:], ag_out[g][s, 0])
                    hth_t[j] = th
                    tl = persist.tile([128, B], bf16, tag=recycle[2 * j + 1],
                                      name=f"htl{j}")
                    nc.sync.dma_start(tl[:], ag_out[g][s, 1])
                    htl_t[j] = tl

            idx_all = [persist.tile([128, EP, 2], fp32, tag=f"idx{m}",
                                    name=f"idx{m}")
                       for m in range(MT)]

            # ---------- phase 4: mm2 --------------------------------------
            # m-tiles in pairs; each (j, m) loads the hT hi/lo stationaries
            # once and streams all 4 column chunks through them
            # (2 LDWEIGHTS per 12 matmuls). W2 is streamed once per m-pair.
            def mm2_epilogue(acc, nch, m):
                # acc: [128, 368] = 8 even positions x (loc|scale) x 23
                bv = b2_t[:, nch * CC:(nch + 1) * CC]
                netE = ep.tile([128, CC], fp32, tag="netE", name="netE")
                nc.vector.tensor_tensor(netE[:], acc[:], bv, Alu.add)
                ng = netE[:].rearrange("p (i s v) -> p i s v", s=2, v=V)
                gmax = ep.tile([128, 8, 2], fp32, tag="gmax", name="gmax")
                nc.vector.tensor_reduce(gmax[:], ng, axis=mybir.AxisListType.X,
                                        op=Alu.max)
                eq = ep.tile([128, 8, 2, V], fp32, tag="eq", name="eq")
                nc.vector.tensor_tensor(
                    eq[:], ng, gmax[:].unsqueeze(3).broadcast_to([128, 8, 2, V]),
                    Alu.is_ge)
                mt = ep.tile([128, 8, 2, V], fp32, tag="mt", name="mt")
                nc.vector.tensor_tensor(
                    mt[:], eq[:],
                    cbi_t[:].unsqueeze(1).unsqueeze(1).broadcast_to(
                        [128, 8, 2, V]), Alu.mult)
                tmax = ep.tile([128, 8, 2], fp32, tag="tmax", name="tmax")
                nc.vector.tensor_reduce(tmax[:], mt[:], axis=mybir.AxisListType.X,
                                        op=Alu.max)
                nc.vector.tensor_scalar(
                    idx_all[m][:, nch * 8:(nch + 1) * 8, :],
                    tmax[:], -1.0, BIG, Alu.mult, Alu.add)

            def mod23(dst_tag, src):
                d = small.tile([128, EP], fp32, tag=dst_tag + "_d",
                               name=dst_tag + "_d")
                nc.vector.tensor_scalar(d[:], src[:], 1.0 / 23.0, -0.49,
                                        Alu.mult, Alu.add)
                q = small.tile([128, EP], fp32, tag=dst_tag + "_q",
                               name=dst_tag + "_q")
                nc.vector.tensor_scalar(q[:], d[:], MAGIC, MAGIC,
                                        Alu.add, Alu.subtract)
                r = small.tile([128, EP], fp32, tag=dst_tag + "_r",
                               name=dst_tag + "_r")
                nc.vector.scalar_tensor_tensor(r[:], q[:], -23.0, src[:],
                                               Alu.mult, Alu.add)
                return r

            def flow_out(m):
                """argmax indices -> modular flow -> one-hot -> DRAM."""
                loc = idx_all[m][:, :, 0]
                scl = idx_all[m][:, :, 1]
                u0 = small.tile([128, EP], fp32, tag="u0", name="u0")
                nc.vector.scalar_tensor_tensor(u0[:], t_tok[m][:], 23.0, loc,
                                               Alu.add, Alu.subtract)
                geu = small.tile([128, EP], fp32, tag="geu", name="geu")
                nc.vector.tensor_single_scalar(geu[:], u0[:], 23.0, Alu.is_ge)
                u = small.tile([128, EP], fp32, tag="u", name="u")
                nc.vector.scalar_tensor_tensor(u[:], geu[:], -23.0, u0[:],
                                               Alu.mult, Alu.add)
                s2 = small.tile([128, EP], fp32, tag="s2", name="s2")
                nc.vector.tensor_tensor(s2[:], scl, scl, Alu.mult)
                s2m = mod23("s2m", s2)
                s4 = small.tile([128, EP], fp32, tag="s4", name="s4")
                nc.vector.tensor_tensor(s4[:], s2m[:], s2m[:], Alu.mult)
                s4m = mod23("s4m", s4)
                s8 = small.tile([128, EP], fp32, tag="s8", name="s8")
                nc.vector.tensor_tensor(s8[:], s4m[:], s4m[:], Alu.mult)
                s8m = mod23("s8m", s8)
                s16 = small.tile([128, EP], fp32, tag="s16", name="s16")
                nc.vector.tensor_tensor(s16[:], s8m[:], s8m[:], Alu.mult)
                s16m = mod23("s16m", s16)
                p1 = small.tile([128, EP], fp32, tag="p1", name="p1")
                nc.vector.tensor_tensor(p1[:], s16m[:], s4m[:], Alu.mult)
                p1m = mod23("p1m", p1)
                p2 = small.tile([128, EP], fp32, tag="p2", name="p2")
                nc.vector.tensor_tensor(p2[:], p1m[:], scl, Alu.mult)
                inv = mod23("inv", p2)
                wprod = small.tile([128, EP], fp32, tag="wprod", name="wprod")
                nc.vector.tensor_tensor(wprod[:], inv[:], u[:], Alu.mult)
                wm = mod23("wm", wprod)
                live = small.tile([128, EP], fp32, tag="live", name="live")
                nc.vector.tensor_single_scalar(live[:], inv[:], 0.5, Alu.is_ge)
                w1p = small.tile([128, EP], fp32, tag="w1p", name="w1p")
                nc.vector.tensor_single_scalar(w1p[:], wm[:], 1.0, Alu.add)
                w2p = small.tile([128, EP], fp32, tag="w2p", name="w2p")
                nc.vector.tensor_tensor(w2p[:], w1p[:], live[:], Alu.mult)
                wfin = small.tile([128, EP], fp32, tag="wfin", name="wfin")
                nc.vector.tensor_single_scalar(wfin[:], w2p[:], -1.0, Alu.add)
                oh = ep.tile([128, EP, V], fp32, tag="oh", name="oh")
                nc.vector.tensor_tensor(
                    oh[:], iota_t[:].unsqueeze(1).broadcast_to([128, EP, V]),
                    wfin[:].unsqueeze(2).broadcast_to([128, EP, V]), Alu.is_equal)
                nc.sync.dma_start(oute[m], oh[:].rearrange("p e v -> p (e v)"))

            # mm2 in two m-pair sweeps (W2 streamed per sweep): pair 0's
            # epilogue + flow run under pair 1's matmuls, shrinking the tail.
            # Each (j, m) loads the hT hi/lo stationaries once for 6 matmuls;
            # 2 chunks x 2 m x 2 pairs = 8 PSUM banks.
            for mp in range(MT // 2):
                ms = (2 * mp, 2 * mp + 1)
                accs = {(mi, nch): ps.tile([128, CC], fp32,
                                           tag=f"p4_{ms[mi]}_{nch}",
                                           name=f"p4_{ms[mi]}_{nch}")
                        for mi in range(2) for nch in range(NCH)}
                for g in range(HM):
                    for s in range(NCORES):
                        j = HM * s + g
                        first = (g == 0 and s == 0)
                        last = (g == HM - 1 and s == NCORES - 1)
                        w2t = w2s.tile([128, 2 * NCH * CC], bf16, tag="w2t",
                                       name="w2t")
                        nc.sync.dma_start(w2t[:], w2a[j])
                        rh = [w2t[:, (2 * n) * CC:(2 * n + 1) * CC]
                              for n in range(NCH)]
                        rl = [w2t[:, (2 * n + 1) * CC:(2 * n + 2) * CC]
                              for n in range(NCH)]
                        for mi, m in enumerate(ms):
                            lh = hth_t[j][:, m * 128:(m + 1) * 128]
                            ll = htl_t[j][:, m * 128:(m + 1) * 128]
                            for nch in range(NCH):
                                a = accs[(mi, nch)]
                                nc.tensor.matmul(a[:], lh, rh[nch],
                                                 start=first, stop=False)
                                nc.tensor.matmul(a[:], lh, rl[nch],
                                                 start=False, stop=False)
                            for nch in range(NCH):
                                a = accs[(mi, nch)]
                                nc.tensor.matmul(a[:], ll, rh[nch],
                                                 start=False, stop=last)
                for mi, m in enumerate(ms):
                    for nch in range(NCH):
                        mm2_epilogue(accs[(mi, nch)], nch, m)
                    flow_out(m)

    nc.compile()
    return nc


def _split_bf16(a):
    hi = a.astype(BF16)
    lo = (a - hi.astype(np.float32)).astype(BF16)
    return hi, lo


def kernel(inputs, mask, W1, b1, W2, b2):
    from concourse.bass_utils import run_bass_kernel_spmd

    if "nc" not in _cache:
        _cache["nc"] = _build()
    nc = _cache["nc"]

    inputs = np.asarray(inputs, np.float32)
    mask = np.asarray(mask, np.float32)
    W1 = np.asarray(W1, np.float32)
    b1 = np.asarray(b1, np.float32)
    W2 = np.asarray(W2, np.float32)
    b2 = np.asarray(b2, np.float32)

    masked = inputs * mask[None, :, :]                    # [B, L, V]
    x_odd = masked[:, 1::2, :].reshape(B, (L // 2) * V)   # [512, 2944]
    xt_np = np.ascontiguousarray(x_odd.T.reshape(KT1, 128, B)).astype(BF16)
    W1_odd = W1.reshape(L, V, H)[1::2].reshape((L // 2) * V, H)

    in_maps = []
    for k in range(NCORES):
        w1s = W1_odd[:, k * HS:(k + 1) * HS]
        w1hi, w1lo = _split_bf16(w1s)
        # odd-position net columns are multiplied by (1-mask)=0 downstream:
        # only the 736 even-position columns of this core's W2 slice matter
        w2sl = W2[:, k * CW:(k + 1) * CW].reshape(H, PS, 2 * V)[:, 0::2, :]
        w2sl = w2sl.reshape(H, CE)
        w2hi, w2lo = _split_bf16(w2sl)
        # interleave (hi,lo) per column chunk: [j, 128, 2*NCH*CC] contiguous
        w2hi = w2hi.reshape(KT2, 128, NCH, CC)
        w2lo = w2lo.reshape(KT2, 128, NCH, CC)
        w2all = np.empty((KT2, 128, 2 * NCH, CC), dtype=BF16)
        w2all[:, :, 0::2] = w2hi
        w2all[:, :, 1::2] = w2lo
        w2all = np.ascontiguousarray(w2all.reshape(KT2, 128, 2 * NCH * CC))
        b2s = b2[k * CW:(k + 1) * CW].reshape(PS, 2 * V)[0::2].reshape(CE)
        cols = slice(32 * k, 32 * k + 32, 2)
        inpe = inputs[:, cols, :].reshape(MT, 128, EP * V)
        in_maps.append({
            "xt": xt_np,
            "w1h": np.ascontiguousarray(w1hi.reshape(KT1, 128, HS)),
            "w1l": np.ascontiguousarray(w1lo.reshape(KT1, 128, HS)),
            "b1s": np.ascontiguousarray(b1[k * HS:(k + 1) * HS].reshape(-1, 128)),
            "w2a": w2all,
            "b2r": np.ascontiguousarray(np.broadcast_to(b2s, (128, CE))),
            "inpe": np.ascontiguousarray(inpe),
        })

    res = run_bass_kernel_spmd(nc, in_maps, core_ids=list(range(NCORES)))
    _cache["last_result"] = res

    out = np.empty((B, L, V), np.float32)
    out[:, 1::2, :] = masked[:, 1::2, :]
    for k in range(NCORES):
        oe = res.results[k]["oute"].reshape(MT, 128, EP, V)
        out[:, 32 * k:32 * k + 32:2, :] = oe.reshape(B, EP, V)
    return out



# revision 6
# speedup vs baseline: 1.2614x; 1.2614x over previous
"""DiscreteBipartiteFlow forward on 8 Trainium2 NeuronCores.

Math (forward pass only):
  masked = mask * inputs                      (mask = 1 at odd l, 0 at even l)
  h   = relu(masked.reshape(B, L*V) @ W1 + b1)
  net = (h @ W2 + b2).reshape(B, L, 2V)
  loc, scale = argmax one-hots of net[..., :V], net[..., V:]
  out[odd l]  = inputs
  out[even l] = onehot((inv(scale) * ((tok - loc) mod V)) mod V), or 0 if scale==0

Sharding (8 cores):
  mm1 tensor-parallel over hidden (core k owns hidden [512k, 512k+512));
  h split to bf16 hi+lo and all-gathered in 4 per-m-tile chunks (pipelined);
  mm2 tensor-parallel over output columns (core k owns positions [32k, 32k+32),
  even ones only); per-core epilogue does argmax + modular flow via table
  lookups; host interleaves position slices.

Schedule design (v2, from trace analysis):
  - big packed partition-major DMAs (~30 total) instead of ~220 small ones:
    the HWDGE issue pipe costs ~0.6us per dma_start regardless of size
  - DMA queues separated so a semaphore wait never blocks a prefetch FIFO:
    sync = xt/W1 stream, vector = W2 prefetch (12MB, SBUF-resident, loaded
    once), scalar = relu + ag_in bounce + AG-gated gather readbacks,
    gpsimd = collective triggers
  - 4 AllGathers fired progressively as each hidden m-tile finishes so the
    serial CC-core pipe (~20us/AG) overlaps mm1+mm2
  - mm2 loops g(gather chunk)-outer, b(batch tile)-mid, s(core)-inner:
    chunk g is consumed just-in-time, and each b's epilogue runs under the
    next b's matmuls; only b=3's epilogue trails the last matmul

Precision: matmuls run as bf16 hi/lo split passes (x one-hot is exact in
bf16: mm1 = 2 passes over W1{hi,lo}; mm2 = 3 passes hh+hl+lh) with fp32
PSUM accumulation -> ~2^-18 operand error, fp32-grade argmax fidelity.
"""

import numpy as np
import ml_dtypes

B, L, V = 512, 256, 23
H = 4096
NCORES = 8
HS = H // NCORES          # 512  hidden shard
HM = HS // 128            # 4    local hidden tiles (m)
PS = L // NCORES          # 32   positions per core
EP = PS // 2              # 16   even positions per core
CW = PS * 2 * V           # 1472 net columns per core (incl. unused odd)
CE = EP * 2 * V           # 736  even-position net columns
CC = CE // 2              # 368  columns per chunk
KT1 = (L // 2) * V // 128  # 23  contraction tiles for mm1
MT = B // 128             # 4    batch tiles (b)
NJ = H // 128             # 32   contraction tiles for mm2

BIG = 64.0
MAGIC = 12582912.0        # 1.5 * 2^23: float32 round-to-int domain
BF16 = ml_dtypes.bfloat16

_cache = {}


def _inv_table():
    return np.array([0] + [pow(a, -1, V) for a in range(1, V)], dtype=np.float32)


def _build():
    import concourse.mybir as mybir
    import concourse.tile as tile
    from concourse import bacc

    fp32 = mybir.dt.float32
    bf16 = mybir.dt.bfloat16
    Alu = mybir.AluOpType
    Act = mybir.ActivationFunctionType

    nc = bacc.Bacc("TRN2", target_bir_lowering=False, debug=False,
                   num_devices=NCORES)

    # ---- per-core inputs (packed partition-major on host) ----
    xtp = nc.dram_tensor("xtp", [128, KT1 * B], bf16, kind="ExternalInput")
    w1p = nc.dram_tensor("w1p", [HM, 128, KT1 * 256], bf16,
                         kind="ExternalInput")   # per (m, k): [hi128 | lo128]
    w2p = nc.dram_tensor("w2p", [128, NJ * CW], bf16,
                         kind="ExternalInput")   # idx=g*8+s: [Rh0|Rl0|Rh1|Rl1]
    b1p = nc.dram_tensor("b1p", [128, HM], fp32, kind="ExternalInput")
    b2r = nc.dram_tensor("b2r", [128, CE], fp32, kind="ExternalInput")
    inpp = nc.dram_tensor("inpp", [128, MT * EP * V], fp32,
                          kind="ExternalInput")
    oute = nc.dram_tensor("oute", [MT, 128, EP * V], fp32,
                          kind="ExternalOutput")

    # ---- constants (baked into the NEFF) ----
    iota_np = np.arange(V, dtype=np.float32)[None, :].repeat(128, 0)
    c_iota = nc.inline_tensor(np.ascontiguousarray(iota_np), name="c_iota")
    c_bi = nc.inline_tensor(np.ascontiguousarray(BIG - iota_np), name="c_bi")
    inv_np = _inv_table()[None, :].repeat(128, 0)
    c_inv = nc.inline_tensor(np.ascontiguousarray(inv_np), name="c_inv")

    with tile.TileContext(nc) as tc:
        with (
            tc.tile_pool(name="persist", bufs=1) as persist,
            tc.tile_pool(name="w1rot", bufs=2) as w1rot,
            tc.tile_pool(name="hthrot", bufs=2) as hthrot,
            tc.tile_pool(name="work", bufs=2) as work,
            tc.tile_pool(name="ework", bufs=1) as ework,
            tc.tile_pool(name="ps", bufs=1, space="PSUM") as ps,
            tc.tile_pool(name="dram", bufs=1, space="DRAM") as dram,
        ):
            # ---------- sync queue: xt + W1 stream (order = issue order) ----
            KA = 12                      # first xt/w1m0 chunk: k tiles 0..11
            xt_t = persist.tile([128, KT1 * B], bf16, tag="xt")
            w1_t = [w1rot.tile([128, KT1 * 256], bf16, tag="w1s",
                               name=f"w1s{m}") for m in range(HM)]
            nc.sync.dma_start(xt_t[:, :KA * B], xtp[:, :KA * B])
            nc.sync.dma_start(w1_t[0][:, :KA * 256], w1p[0][:, :KA * 256])
            nc.sync.dma_start(xt_t[:, KA * B:], xtp[:, KA * B:])
            nc.sync.dma_start(w1_t[0][:, KA * 256:], w1p[0][:, KA * 256:])
            nc.sync.dma_start(w1_t[1][:], w1p[1])
            b1_t = persist.tile([128, HM], fp32, tag="b1")
            nc.sync.dma_start(b1_t[:], b1p[:])
            iota_t = persist.tile([128, V], fp32, tag="iota")
            nc.sync.dma_start(iota_t[:], c_iota[:])
            inp_t = persist.tile([128, MT * EP * V], fp32, tag="inpp")
            nc.sync.dma_start(inp_t[:], inpp[:])
            cbi_t = persist.tile([128, V], fp32, tag="cbi")
            nc.sync.dma_start(cbi_t[:], c_bi[:])
            cinv_t = persist.tile([128, V], fp32, tag="cinv")
            nc.sync.dma_start(cinv_t[:], c_inv[:])
            # w1 m2/m3 reuse m0/m1 buffers; their issue blocks sync until the
            # m0/m1 k-loops retire -- everything below here on sync is slack
            nc.sync.dma_start(w1_t[2][:], w1p[2])
            nc.sync.dma_start(w1_t[3][:], w1p[3])
            b2_t = persist.tile([128, CE], fp32, tag="b2")
            nc.sync.dma_start(b2_t[:], b2r[:])

            # ---------- scalar queue: W2 prefetch (12 MB, SBUF-resident) ----
            # issues complete before relu m0's semaphore wait blocks scalar
            w2_t = persist.tile([128, NJ * CW], bf16, tag="w2")
            W2CH = 4 * CW                # 4 j-tiles per chunk
            for i in range(8):
                nc.scalar.dma_start(w2_t[:, i * W2CH:(i + 1) * W2CH],
                                    w2p[:, i * W2CH:(i + 1) * W2CH])

            # token index per batch tile (vector; runs during mm1)
            tok_t = []
            for b in range(MT):
                tmp = work.tile([128, EP, V], fp32, tag="tokmul")
                nc.vector.tensor_tensor(
                    tmp[:],
                    inp_t[:, b * EP * V:(b + 1) * EP * V].rearrange(
                        "p (e v) -> p e v", v=V),
                    iota_t[:].unsqueeze(1).broadcast_to([128, EP, V]),
                    Alu.mult)
                tk = persist.tile([128, EP], fp32, tag=f"tok{b}")
                nc.vector.tensor_reduce(tk[:], tmp[:],
                                        axis=mybir.AxisListType.X, op=Alu.add)
                tok_t.append(tk)

            # ---------- collective buffers ----------
            agin = [dram.tile([128, 2 * B], bf16, tag=f"agin{m}",
                              name=f"agin{m}") for m in range(HM)]
            agout = [dram.tile([NCORES, 128, 2 * B], bf16, tag=f"agout{m}",
                               name=f"agout{m}", addr_space="Shared")
                     for m in range(HM)]

            hth = []

            def emit_rb(g):
                t = hthrot.tile([128, NCORES * 2 * B], bf16, tag="hth",
                                name=f"hth{g}")
                nc.scalar.dma_start(
                    t[:].rearrange("p (s c) -> p s c", s=NCORES),
                    agout[g][:].rearrange("s p c -> p s c"))
                hth.append(t)

            # ---------- phase 1: mm1 (m-outer, k-inner, hi/lo interleaved) --
            for m in range(HM):
                acc = ps.tile([128, B], fp32, tag=f"pm{m}", name=f"pm{m}")
                for k in range(KT1):
                    nc.tensor.matmul(acc[:], w1_t[m][:, k * 256:k * 256 + 128],
                                     xt_t[:, k * B:(k + 1) * B],
                                     start=(k == 0), stop=False)
                    nc.tensor.matmul(acc[:],
                                     w1_t[m][:, k * 256 + 128:(k + 1) * 256],
                                     xt_t[:, k * B:(k + 1) * B],
                                     start=False, stop=(k == KT1 - 1))
                hf = work.tile([128, B], fp32, tag="hf")
                nc.scalar.activation(hf[:], acc[:], Act.Relu,
                                     bias=b1_t[:, m:m + 1], scale=1.0)
                hs = work.tile([128, 2 * B], bf16, tag="hs")
                nc.vector.tensor_copy(hs[:, 0:B], hf[:])
                nc.vector.tensor_sub(hs[:, B:2 * B], hf[:], hs[:, 0:B])
                nc.scalar.dma_start(agin[m][:], hs[:])
                nc.gpsimd.collective_compute(
                    "AllGather", Alu.bypass,
                    replica_groups=[list(range(NCORES))],
                    ins=[agin[m].opt()], outs=[agout[m].opt()],
                )
                # gather readbacks ride the scalar queue, interleaved so each
                # is issued as early as its AG can possibly be done
                if m == 1:
                    emit_rb(0)
                elif m == 2:
                    emit_rb(1)
                elif m == 3:
                    emit_rb(2)
                    emit_rb(3)

            # ---------- phase 2: mm2 (g-outer, b-mid, s-inner) -------------
            tagmap = {(0, 0): "pc00", (0, 1): "pc01",
                      (1, 0): "pc10", (1, 1): "pc11",
                      (2, 0): "pm0", (2, 1): "pm1",
                      (3, 0): "pm2", (3, 1): "pm3"}
            accs = {(b, ch): ps.tile([128, CC], fp32, tag=tagmap[(b, ch)],
                                     name=f"pc{b}{ch}")
                    for b in range(MT) for ch in range(2)}

            idx_t = {}

            def mm2_epilogue(b, ch):
                acc = accs[(b, ch)]
                bv = b2_t[:, ch * CC:(ch + 1) * CC]
                netE = ework.tile([128, CC], fp32, tag="netE", name="netE")
                nc.vector.tensor_tensor(netE[:], acc[:], bv, Alu.add)
                ng = netE[:].rearrange("p (i s v) -> p i s v", s=2, v=V)
                gmax = ework.tile([128, 8, 2], fp32, tag="gmax", name="gmax")
                nc.vector.tensor_reduce(gmax[:], ng, axis=mybir.AxisListType.X,
                                        op=Alu.max)
                eq = ework.tile([128, 8, 2, V], fp32, tag="eq", name="eq")
                nc.vector.tensor_tensor(
                    eq[:], ng,
                    gmax[:].unsqueeze(3).broadcast_to([128, 8, 2, V]),
                    Alu.is_ge)
                mt = ework.tile([128, 8, 2, V], fp32, tag="mt", name="mt")
                nc.vector.tensor_tensor(
                    mt[:], eq[:],
                    cbi_t[:].unsqueeze(1).unsqueeze(1).broadcast_to(
                        [128, 8, 2, V]), Alu.mult)
                tmax = ework.tile([128, 8, 2], fp32, tag="tmax", name="tmax")
                nc.vector.tensor_reduce(tmax[:], mt[:],
                                        axis=mybir.AxisListType.X, op=Alu.max)
                nc.vector.tensor_scalar(
                    idx_t[b][:, ch * 8:(ch + 1) * 8, :],
                    tmax[:], -1.0, BIG, Alu.mult, Alu.add)

            def flow_out(b):
                """argmax indices -> modular flow via table lookups -> DRAM."""
                loc = idx_t[b][:, :, 0]
                scl = idx_t[b][:, :, 1]
                u0 = ework.tile([128, EP], fp32, tag="u0", name="u0")
                nc.vector.scalar_tensor_tensor(u0[:], tok_t[b][:], float(V),
                                               loc, Alu.add, Alu.subtract)
                geu = ework.tile([128, EP], fp32, tag="geu", name="geu")
                nc.vector.tensor_single_scalar(geu[:], u0[:], float(V),
                                               Alu.is_ge)
                u = ework.tile([128, EP], fp32, tag="u", name="u")
                nc.vector.scalar_tensor_tensor(u[:], geu[:], -float(V), u0[:],
                                               Alu.mult, Alu.add)
                # inv = invtab[scl] via one-hot contraction
                eqs = ework.tile([128, EP, V], fp32, tag="eqs", name="eqs")
                nc.vector.tensor_tensor(
                    eqs[:],
                    iota_t[:].unsqueeze(1).broadcast_to([128, EP, V]),
                    scl.unsqueeze(2).broadcast_to([128, EP, V]),
                    Alu.is_equal)
                tmpi = ework.tile([128, EP, V], fp32, tag="tmpi", name="tmpi")
                nc.vector.tensor_tensor(
                    tmpi[:], eqs[:],
                    cinv_t[:].unsqueeze(1).broadcast_to([128, EP, V]),
                    Alu.mult)
                inv = ework.tile([128, EP], fp32, tag="inv", name="inv")
                nc.vector.tensor_reduce(inv[:], tmpi[:],
                                        axis=mybir.AxisListType.X, op=Alu.add)
                # w = (inv * u) mod 23  (product <= 484, exact in fp32)
                pr = ework.tile([128, EP], fp32, tag="pr", name="pr")
                nc.vector.tensor_tensor(pr[:], inv[:], u[:], Alu.mult)
                d = ework.tile([128, EP], fp32, tag="md", name="md")
                nc.vector.tensor_scalar(d[:], pr[:], 1.0 / V, -0.49,
                                        Alu.mult, Alu.add)
                q = ework.tile([128, EP], fp32, tag="mq", name="mq")
                nc.vector.tensor_scalar(q[:], d[:], MAGIC, MAGIC,
                                        Alu.add, Alu.subtract)
                w = ework.tile([128, EP], fp32, tag="mw", name="mw")
                nc.vector.scalar_tensor_tensor(w[:], q[:], -float(V), pr[:],
                                               Alu.mult, Alu.add)
                live = ework.tile([128, EP], fp32, tag="live", name="live")
                nc.vector.tensor_single_scalar(live[:], inv[:], 0.5, Alu.is_ge)
                w1x = ework.tile([128, EP], fp32, tag="w1x", name="w1x")
                nc.vector.tensor_single_scalar(w1x[:], w[:], 1.0, Alu.add)
                w2x = ework.tile([128, EP], fp32, tag="w2x", name="w2x")
                nc.vector.tensor_tensor(w2x[:], w1x[:], live[:], Alu.mult)
                wfin = ework.tile([128, EP], fp32, tag="wfin", name="wfin")
                nc.vector.tensor_single_scalar(wfin[:], w2x[:], -1.0, Alu.add)
                oh = ework.tile([128, EP, V], fp32, tag="oh", name="oh")
                nc.vector.tensor_tensor(
                    oh[:], iota_t[:].unsqueeze(1).broadcast_to([128, EP, V]),
                    wfin[:].unsqueeze(2).broadcast_to([128, EP, V]),
                    Alu.is_equal)
                nc.sync.dma_start(oute[b], oh[:].rearrange("p e v -> p (e v)"))

            for b in range(MT):
                idx_t[b] = persist.tile([128, EP, 2], fp32, tag=f"idx{b}",
                                        name=f"idx{b}")

            for g in range(HM):
                for b in range(MT):
                    a0, a1 = accs[(b, 0)], accs[(b, 1)]
                    for s in range(NCORES):
                        idx = g * NCORES + s
                        w2j = w2_t[:, idx * CW:(idx + 1) * CW]
                        base = s * 2 * B
                        hi = hth[g][:, base + b * 128:base + b * 128 + 128]
                        lo = hth[g][:, base + B + b * 128:
                                    base + B + b * 128 + 128]
                        first = (g == 0 and s == 0)
                        last = (g == HM - 1 and s == NCORES - 1)
                        nc.tensor.matmul(a0[:], hi, w2j[:, 0:CC],
                                         start=first, stop=False)
                        nc.tensor.matmul(a1[:], hi, w2j[:, 2 * CC:3 * CC],
                                         start=first, stop=False)
                        nc.tensor.matmul(a0[:], hi, w2j[:, CC:2 * CC],
                                         start=False, stop=False)
                        nc.tensor.matmul(a1[:], hi, w2j[:, 3 * CC:4 * CC],
                                         start=False, stop=False)
                        nc.tensor.matmul(a0[:], lo, w2j[:, 0:CC],
                                         start=False, stop=last)
                        nc.tensor.matmul(a1[:], lo, w2j[:, 2 * CC:3 * CC],
                                         start=False, stop=last)
                    if g == HM - 1:
                        mm2_epilogue(b, 0)
                        mm2_epilogue(b, 1)
                        flow_out(b)

    nc.compile()
    return nc


def _split_bf16(a):
    hi = a.astype(BF16)
    lo = (a - hi.astype(np.float32)).astype(BF16)
    return hi, lo


def kernel(inputs, mask, W1, b1, W2, b2):
    from concourse.bass_utils import run_bass_kernel_spmd

    if "nc" not in _cache:
        _cache["nc"] = _build()
    nc = _cache["nc"]

    inputs = np.asarray(inputs, np.float32)
    mask = np.asarray(mask, np.float32)
    W1 = np.asarray(W1, np.float32)
    b1 = np.asarray(b1, np.float32)
    W2 = np.asarray(W2, np.float32)
    b2 = np.asarray(b2, np.float32)

    masked = inputs * mask[None, :, :]                    # [B, L, V]
    x_odd = masked[:, 1::2, :].reshape(B, (L // 2) * V)   # [512, 2944]
    # xtp [128, KT1*B]: partition p = contraction row within k-tile
    xtp = np.ascontiguousarray(
        x_odd.T.reshape(KT1, 128, B).transpose(1, 0, 2).reshape(128, KT1 * B)
    ).astype(BF16)
    W1_odd = W1.reshape(L, V, H)[1::2].reshape((L // 2) * V, H)

    in_maps = []
    for c in range(NCORES):
        w1s = W1_odd[:, c * HS:(c + 1) * HS]              # [2944, 512]
        w1hi, w1lo = _split_bf16(w1s)
        hi_t = w1hi.reshape(KT1, 128, HM, 128)
        lo_t = w1lo.reshape(KT1, 128, HM, 128)
        w1pk = np.empty((HM, 128, KT1, 2, 128), dtype=BF16)
        w1pk[:, :, :, 0, :] = hi_t.transpose(2, 1, 0, 3)
        w1pk[:, :, :, 1, :] = lo_t.transpose(2, 1, 0, 3)
        w1pn = np.ascontiguousarray(w1pk.reshape(HM, 128, KT1 * 256))

        # W2 even-position columns, packed in mm2 consumption order
        W2e = W2[:, c * CW:(c + 1) * CW].reshape(H, PS, 2 * V)[:, 0::2]
        W2e = W2e.reshape(H, CE)
        w2hi, w2lo = _split_bf16(W2e)
        w2pn = np.empty((128, NJ * CW), dtype=BF16)
        for g in range(HM):
            for s in range(NCORES):
                idx = g * NCORES + s
                r0 = s * HS + g * 128
                blk = np.empty((128, 4, CC), dtype=BF16)
                blk[:, 0] = w2hi[r0:r0 + 128, 0:CC]
                blk[:, 1] = w2lo[r0:r0 + 128, 0:CC]
                blk[:, 2] = w2hi[r0:r0 + 128, CC:2 * CC]
                blk[:, 3] = w2lo[r0:r0 + 128, CC:2 * CC]
                w2pn[:, idx * CW:(idx + 1) * CW] = blk.reshape(128, CW)

        b1pn = np.ascontiguousarray(
            b1[c * HS:(c + 1) * HS].reshape(HM, 128).T)
        b2s = b2[c * CW:(c + 1) * CW].reshape(PS, 2 * V)[0::2].reshape(CE)
        cols = slice(32 * c, 32 * c + 32, 2)
        inpe = inputs[:, cols, :].reshape(MT, 128, EP * V)
        inpp = np.ascontiguousarray(
            inpe.transpose(1, 0, 2).reshape(128, MT * EP * V))
        in_maps.append({
            "xtp": xtp,
            "w1p": w1pn,
            "w2p": np.ascontiguousarray(w2pn),
            "b1p": b1pn,
            "b2r": np.ascontiguousarray(np.broadcast_to(b2s, (128, CE))),
            "inpp": inpp,
        })

    res = run_bass_kernel_spmd(nc, in_maps, core_ids=list(range(NCORES)))
    _cache["last_result"] = res

    out = np.empty((B, L, V), np.float32)
    out[:, 1::2, :] = masked[:, 1::2, :]
    for c in range(NCORES):
        oe = res.results[c]["oute"].reshape(MT, 128, EP, V)
        out[:, 32 * c:32 * c + 32:2, :] = oe.reshape(B, EP, V)
    return out


# revision 7
# speedup vs baseline: 1.3386x; 1.0612x over previous
"""DiscreteBipartiteFlow forward on 8 Trainium2 NeuronCores.

Math (forward pass only):
  masked = mask * inputs                      (mask = 1 at odd l, 0 at even l)
  h   = relu(masked.reshape(B, L*V) @ W1 + b1)
  net = (h @ W2 + b2).reshape(B, L, 2V)
  loc, scale = argmax one-hots of net[..., :V], net[..., V:]
  out[odd l]  = inputs
  out[even l] = onehot((inv(scale) * ((tok - loc) mod V)) mod V), or 0 if scale==0

Sharding (8 cores):
  mm1 tensor-parallel over hidden (core k owns hidden [512k, 512k+512));
  h split to bf16 hi+lo and all-gathered in 4 per-m-tile chunks (pipelined);
  mm2 tensor-parallel over output columns (core k owns positions [32k, 32k+32),
  even ones only); per-core epilogue does argmax + modular flow via table
  lookups; host interleaves position slices.

Schedule design (v2, from trace analysis):
  - big packed partition-major DMAs (~30 total) instead of ~220 small ones:
    the HWDGE issue pipe costs ~0.6us per dma_start regardless of size
  - DMA queues separated so a semaphore wait never blocks a prefetch FIFO:
    sync = xt/W1 stream, vector = W2 prefetch (12MB, SBUF-resident, loaded
    once), scalar = relu + ag_in bounce + AG-gated gather readbacks,
    gpsimd = collective triggers
  - 4 AllGathers fired progressively as each hidden m-tile finishes so the
    serial CC-core pipe (~20us/AG) overlaps mm1+mm2
  - mm2 loops g(gather chunk)-outer, b(batch tile)-mid, s(core)-inner:
    chunk g is consumed just-in-time, and each b's epilogue runs under the
    next b's matmuls; only b=3's epilogue trails the last matmul

Precision: matmuls run as bf16 hi/lo split passes (x one-hot is exact in
bf16: mm1 = 2 passes over W1{hi,lo}; mm2 = 3 passes hh+hl+lh) with fp32
PSUM accumulation -> ~2^-18 operand error, fp32-grade argmax fidelity.
"""

import numpy as np
import ml_dtypes

B, L, V = 512, 256, 23
H = 4096
NCORES = 8
HS = H // NCORES          # 512  hidden shard
HM = HS // 128            # 4    local hidden tiles (m)
PS = L // NCORES          # 32   positions per core
EP = PS // 2              # 16   even positions per core
CW = PS * 2 * V           # 1472 net columns per core (incl. unused odd)
CE = EP * 2 * V           # 736  even-position net columns
CC = CE // 2              # 368  columns per chunk
KT1 = (L // 2) * V // 128  # 23  contraction tiles for mm1
MT = B // 128             # 4    batch tiles (b)
NJ = H // 128             # 32   contraction tiles for mm2

BIG = 64.0
MAGIC = 12582912.0        # 1.5 * 2^23: float32 round-to-int domain
BF16 = ml_dtypes.bfloat16

_cache = {}


def _inv_table():
    return np.array([0] + [pow(a, -1, V) for a in range(1, V)], dtype=np.float32)


def _build():
    import concourse.mybir as mybir
    import concourse.tile as tile
    from concourse import bacc

    fp32 = mybir.dt.float32
    bf16 = mybir.dt.bfloat16
    Alu = mybir.AluOpType
    Act = mybir.ActivationFunctionType

    nc = bacc.Bacc("TRN2", target_bir_lowering=False, debug=False,
                   num_devices=NCORES)

    # ---- per-core inputs (packed partition-major on host) ----
    xtp = nc.dram_tensor("xtp", [128, KT1 * B], bf16, kind="ExternalInput")
    w1p = nc.dram_tensor("w1p", [HM, 128, KT1 * 256], bf16,
                         kind="ExternalInput")   # per (m, k): [hi128 | lo128]
    w2p = nc.dram_tensor("w2p", [128, NJ * CW], bf16,
                         kind="ExternalInput")   # idx=g*8+s: [Rh0|Rl0|Rh1|Rl1]
    b1p = nc.dram_tensor("b1p", [128, HM], fp32, kind="ExternalInput")
    b2r = nc.dram_tensor("b2r", [128, CE], fp32, kind="ExternalInput")
    inpp = nc.dram_tensor("inpp", [128, MT * EP * V], fp32,
                          kind="ExternalInput")
    oute = nc.dram_tensor("oute", [MT, 128, EP * V], fp32,
                          kind="ExternalOutput")

    # ---- constants (baked into the NEFF) ----
    iota_np = np.arange(V, dtype=np.float32)[None, :].repeat(128, 0)
    c_iota = nc.inline_tensor(np.ascontiguousarray(iota_np), name="c_iota")
    c_bi = nc.inline_tensor(np.ascontiguousarray(BIG - iota_np), name="c_bi")
    inv_np = _inv_table()[None, :].repeat(128, 0)
    c_inv = nc.inline_tensor(np.ascontiguousarray(inv_np), name="c_inv")

    with tile.TileContext(nc) as tc:
        with (
            tc.tile_pool(name="persist", bufs=1) as persist,
            tc.tile_pool(name="w1rot", bufs=2) as w1rot,
            tc.tile_pool(name="hthrot", bufs=4) as hthrot,
            tc.tile_pool(name="work", bufs=2) as work,
            tc.tile_pool(name="ework", bufs=1) as ework,
            tc.tile_pool(name="ps", bufs=1, space="PSUM") as ps,
            tc.tile_pool(name="dram", bufs=1, space="DRAM") as dram,
        ):
            # ---------- sync queue: xt + W1 + W2 stream (order = drain order)
            KA = 12                      # first xt/w1m0 chunk: k tiles 0..11
            xt_t = persist.tile([128, KT1 * B], bf16, tag="xt")
            w1_t = [w1rot.tile([128, KT1 * 256], bf16, tag="w1s",
                               name=f"w1s{m}") for m in range(HM)]
            w2_t = persist.tile([128, NJ * CW], bf16, tag="w2")
            W2CH = 4 * CW                # 4 j-tiles per chunk
            nc.sync.dma_start(xt_t[:, :KA * B], xtp[:, :KA * B])
            nc.sync.dma_start(w1_t[0][:, :KA * 256], w1p[0][:, :KA * 256])
            nc.sync.dma_start(xt_t[:, KA * B:], xtp[:, KA * B:])
            nc.sync.dma_start(w1_t[0][:, KA * 256:], w1p[0][:, KA * 256:])
            nc.sync.dma_start(w1_t[1][:], w1p[1])
            b1_t = persist.tile([128, HM], fp32, tag="b1")
            nc.sync.dma_start(b1_t[:], b1p[:])
            iota_t = persist.tile([128, V], fp32, tag="iota")
            nc.sync.dma_start(iota_t[:], c_iota[:])
            inp_t = persist.tile([128, MT * EP * V], fp32, tag="inpp")
            nc.sync.dma_start(inp_t[:], inpp[:])
            cbi_t = persist.tile([128, V], fp32, tag="cbi")
            nc.sync.dma_start(cbi_t[:], c_bi[:])
            cinv_t = persist.tile([128, V], fp32, tag="cinv")
            nc.sync.dma_start(cinv_t[:], c_inv[:])
            # w1 m2/m3 reuse m0/m1 buffers (WAR-gated); W2 chunks fill the
            # remaining sync-ring slack, all landing before mm2 consumes them
            nc.sync.dma_start(w1_t[2][:], w1p[2])
            nc.sync.dma_start(w2_t[:, 0 * W2CH:1 * W2CH], w2p[:, 0 * W2CH:1 * W2CH])
            nc.sync.dma_start(w2_t[:, 1 * W2CH:2 * W2CH], w2p[:, 1 * W2CH:2 * W2CH])
            nc.sync.dma_start(w1_t[3][:], w1p[3])
            for i in range(2, 8):
                nc.sync.dma_start(w2_t[:, i * W2CH:(i + 1) * W2CH],
                                  w2p[:, i * W2CH:(i + 1) * W2CH])
            b2_t = persist.tile([128, CE], fp32, tag="b2")
            nc.sync.dma_start(b2_t[:], b2r[:])

            # token index per batch tile (vector; runs during mm1)
            tok_t = []
            for b in range(MT):
                tmp = ework.tile([128, EP, V], fp32, tag="tokmul")
                nc.vector.tensor_tensor(
                    tmp[:],
                    inp_t[:, b * EP * V:(b + 1) * EP * V].rearrange(
                        "p (e v) -> p e v", v=V),
                    iota_t[:].unsqueeze(1).broadcast_to([128, EP, V]),
                    Alu.mult)
                tk = persist.tile([128, EP], fp32, tag=f"tok{b}")
                nc.vector.tensor_reduce(tk[:], tmp[:],
                                        axis=mybir.AxisListType.X, op=Alu.add)
                tok_t.append(tk)

            # ---------- collective buffers: one AG per (m, batch-half) ------
            NC2 = 2 * HM                 # 8 gather chunks
            HB = B // 2                  # 256 batch rows per half
            agin = [dram.tile([128, 2 * HB], bf16, tag=f"agin{c}",
                              name=f"agin{c}") for c in range(NC2)]
            agout = [dram.tile([NCORES, 128, 2 * HB], bf16, tag=f"agout{c}",
                               name=f"agout{c}", addr_space="Shared")
                     for c in range(NC2)]

            hth = []

            def emit_rb(c):
                t = hthrot.tile([128, NCORES * 2 * HB], bf16, tag="hth",
                                name=f"hth{c}")
                nc.gpsimd.dma_start(
                    t[:].rearrange("p (s c) -> p s c", s=NCORES),
                    agout[c][:].rearrange("s p c -> p s c"))
                hth.append(t)

            # rb emission points on the gpsimd FIFO: late enough that the
            # AG-completion wait never delays a pending trigger
            rb_after = {3: [0], 4: [1], 5: [2], 6: [3], 7: [4, 5, 6, 7]}

            # ---------- phase 1: mm1, N=256 half-batch k-loops --------------
            for m in range(HM):
                acc = ps.tile([128, B], fp32, tag=f"pm{m}", name=f"pm{m}")
                for bh in range(2):
                    c = 2 * m + bh
                    lo_c, hi_c = bh * HB, (bh + 1) * HB
                    for k in range(KT1):
                        xs = xt_t[:, k * B + lo_c:k * B + hi_c]
                        nc.tensor.matmul(acc[:, lo_c:hi_c],
                                         w1_t[m][:, k * 256:k * 256 + 128],
                                         xs, start=(k == 0), stop=False)
                        nc.tensor.matmul(acc[:, lo_c:hi_c],
                                         w1_t[m][:, k * 256 + 128:(k + 1) * 256],
                                         xs, start=False, stop=(k == KT1 - 1))
                    hf = ework.tile([128, HB], fp32, tag="hf")
                    nc.scalar.activation(hf[:], acc[:, lo_c:hi_c], Act.Relu,
                                         bias=b1_t[:, m:m + 1], scale=1.0)
                    hs = ework.tile([128, 2 * HB], bf16, tag="hs")
                    nc.vector.tensor_copy(hs[:, 0:HB], hf[:])
                    nc.vector.tensor_sub(hs[:, HB:2 * HB], hf[:], hs[:, 0:HB])
                    nc.scalar.dma_start(agin[c][:], hs[:])
                    nc.gpsimd.collective_compute(
                        "AllGather", Alu.bypass,
                        replica_groups=[list(range(NCORES))],
                        ins=[agin[c].opt()], outs=[agout[c].opt()],
                    )
                    for r in rb_after.get(c, []):
                        emit_rb(r)

            # ---------- phase 2: mm2 (g-outer, b-mid, s-inner) -------------
            tagmap = {(0, 0): "pc00", (0, 1): "pc01",
                      (1, 0): "pc10", (1, 1): "pc11",
                      (2, 0): "pm0", (2, 1): "pm1",
                      (3, 0): "pm2", (3, 1): "pm3"}
            accs = {(b, ch): ps.tile([128, CC], fp32, tag=tagmap[(b, ch)],
                                     name=f"pc{b}{ch}")
                    for b in range(MT) for ch in range(2)}

            idx_t = {}

            def mm2_epilogue(b, ch):
                acc = accs[(b, ch)]
                bv = b2_t[:, ch * CC:(ch + 1) * CC]
                netE = ework.tile([128, CC], fp32, tag="netE", name="netE")
                nc.vector.tensor_tensor(netE[:], acc[:], bv, Alu.add)
                ng = netE[:].rearrange("p (i s v) -> p i s v", s=2, v=V)
                gmax = ework.tile([128, 8, 2], fp32, tag="gmax", name="gmax")
                nc.vector.tensor_reduce(gmax[:], ng, axis=mybir.AxisListType.X,
                                        op=Alu.max)
                eq = ework.tile([128, 8, 2, V], fp32, tag="eq", name="eq")
                nc.vector.tensor_tensor(
                    eq[:], ng,
                    gmax[:].unsqueeze(3).broadcast_to([128, 8, 2, V]),
                    Alu.is_ge)
                mt = ework.tile([128, 8, 2, V], fp32, tag="mt", name="mt")
                nc.vector.tensor_tensor(
                    mt[:], eq[:],
                    cbi_t[:].unsqueeze(1).unsqueeze(1).broadcast_to(
                        [128, 8, 2, V]), Alu.mult)
                tmax = ework.tile([128, 8, 2], fp32, tag="tmax", name="tmax")
                nc.vector.tensor_reduce(tmax[:], mt[:],
                                        axis=mybir.AxisListType.X, op=Alu.max)
                nc.vector.tensor_scalar(
                    idx_t[b][:, ch * 8:(ch + 1) * 8, :],
                    tmax[:], -1.0, BIG, Alu.mult, Alu.add)

            def flow_out(b):
                """argmax indices -> modular flow via table lookups -> DRAM."""
                loc = idx_t[b][:, :, 0]
                scl = idx_t[b][:, :, 1]
                u0 = ework.tile([128, EP], fp32, tag="u0", name="u0")
                nc.vector.scalar_tensor_tensor(u0[:], tok_t[b][:], float(V),
                                               loc, Alu.add, Alu.subtract)
                geu = ework.tile([128, EP], fp32, tag="geu", name="geu")
                nc.vector.tensor_single_scalar(geu[:], u0[:], float(V),
                                               Alu.is_ge)
                u = ework.tile([128, EP], fp32, tag="u", name="u")
                nc.vector.scalar_tensor_tensor(u[:], geu[:], -float(V), u0[:],
                                               Alu.mult, Alu.add)
                # inv = invtab[scl] via one-hot contraction
                eqs = ework.tile([128, EP, V], fp32, tag="eqs", name="eqs")
                nc.vector.tensor_tensor(
                    eqs[:],
                    iota_t[:].unsqueeze(1).broadcast_to([128, EP, V]),
                    scl.unsqueeze(2).broadcast_to([128, EP, V]),
                    Alu.is_equal)
                tmpi = ework.tile([128, EP, V], fp32, tag="tmpi", name="tmpi")
                nc.vector.tensor_tensor(
                    tmpi[:], eqs[:],
                    cinv_t[:].unsqueeze(1).broadcast_to([128, EP, V]),
                    Alu.mult)
                inv = ework.tile([128, EP], fp32, tag="inv", name="inv")
                nc.vector.tensor_reduce(inv[:], tmpi[:],
                                        axis=mybir.AxisListType.X, op=Alu.add)
                # w = (inv * u) mod 23  (product <= 484, exact in fp32)
                pr = ework.tile([128, EP], fp32, tag="pr", name="pr")
                nc.vector.tensor_tensor(pr[:], inv[:], u[:], Alu.mult)
                d = ework.tile([128, EP], fp32, tag="md", name="md")
                nc.vector.tensor_scalar(d[:], pr[:], 1.0 / V, -0.49,
                                        Alu.mult, Alu.add)
                q = ework.tile([128, EP], fp32, tag="mq", name="mq")
                nc.vector.tensor_scalar(q[:], d[:], MAGIC, MAGIC,
                                        Alu.add, Alu.subtract)
                w = ework.tile([128, EP], fp32, tag="mw", name="mw")
                nc.vector.scalar_tensor_tensor(w[:], q[:], -float(V), pr[:],
                                               Alu.mult, Alu.add)
                live = ework.tile([128, EP], fp32, tag="live", name="live")
                nc.vector.tensor_single_scalar(live[:], inv[:], 0.5, Alu.is_ge)
                w1x = ework.tile([128, EP], fp32, tag="w1x", name="w1x")
                nc.vector.tensor_single_scalar(w1x[:], w[:], 1.0, Alu.add)
                w2x = ework.tile([128, EP], fp32, tag="w2x", name="w2x")
                nc.vector.tensor_tensor(w2x[:], w1x[:], live[:], Alu.mult)
                wfin = ework.tile([128, EP], fp32, tag="wfin", name="wfin")
                nc.vector.tensor_single_scalar(wfin[:], w2x[:], -1.0, Alu.add)
                oh = ework.tile([128, EP, V], fp32, tag="oh", name="oh")
                nc.vector.tensor_tensor(
                    oh[:], iota_t[:].unsqueeze(1).broadcast_to([128, EP, V]),
                    wfin[:].unsqueeze(2).broadcast_to([128, EP, V]),
                    Alu.is_equal)
                nc.sync.dma_start(oute[b], oh[:].rearrange("p e v -> p (e v)"))

            for b in range(MT):
                idx_t[b] = persist.tile([128, EP, 2], fp32, tag=f"idx{b}",
                                        name=f"idx{b}")

            for g in range(HM):
                for b in range(MT):
                    a0, a1 = accs[(b, 0)], accs[(b, 1)]
                    hthc = hth[2 * g + b // 2]
                    bo = (b % 2) * 128
                    for s in range(NCORES):
                        idx = g * NCORES + s
                        w2j = w2_t[:, idx * CW:(idx + 1) * CW]
                        base = s * 2 * HB
                        hi = hthc[:, base + bo:base + bo + 128]
                        lo = hthc[:, base + HB + bo:base + HB + bo + 128]
                        first = (g == 0 and s == 0)
                        last = (g == HM - 1 and s == NCORES - 1)
                        nc.tensor.matmul(a0[:], hi, w2j[:, 0:CC],
                                         start=first, stop=False)
                        nc.tensor.matmul(a1[:], hi, w2j[:, 2 * CC:3 * CC],
                                         start=first, stop=False)
                        nc.tensor.matmul(a0[:], hi, w2j[:, CC:2 * CC],
                                         start=False, stop=False)
                        nc.tensor.matmul(a1[:], hi, w2j[:, 3 * CC:4 * CC],
                                         start=False, stop=False)
                        nc.tensor.matmul(a0[:], lo, w2j[:, 0:CC],
                                         start=False, stop=last)
                        nc.tensor.matmul(a1[:], lo, w2j[:, 2 * CC:3 * CC],
                                         start=False, stop=last)
                    if g == HM - 1:
                        mm2_epilogue(b, 0)
                        mm2_epilogue(b, 1)
                        flow_out(b)

    nc.compile()
    return nc


def _split_bf16(a):
    hi = a.astype(BF16)
    lo = (a - hi.astype(np.float32)).astype(BF16)
    return hi, lo


def kernel(inputs, mask, W1, b1, W2, b2):
    from concourse.bass_utils import run_bass_kernel_spmd

    if "nc" not in _cache:
        _cache["nc"] = _build()
    nc = _cache["nc"]

    inputs = np.asarray(inputs, np.float32)
    mask = np.asarray(mask, np.float32)
    W1 = np.asarray(W1, np.float32)
    b1 = np.asarray(b1, np.float32)
    W2 = np.asarray(W2, np.float32)
    b2 = np.asarray(b2, np.float32)

    masked = inputs * mask[None, :, :]                    # [B, L, V]
    x_odd = masked[:, 1::2, :].reshape(B, (L // 2) * V)   # [512, 2944]
    # xtp [128, KT1*B]: partition p = contraction row within k-tile
    xtp = np.ascontiguousarray(
        x_odd.T.reshape(KT1, 128, B).transpose(1, 0, 2).reshape(128, KT1 * B)
    ).astype(BF16)
    W1_odd = W1.reshape(L, V, H)[1::2].reshape((L // 2) * V, H)

    in_maps = []
    for c in range(NCORES):
        w1s = W1_odd[:, c * HS:(c + 1) * HS]              # [2944, 512]
        w1hi, w1lo = _split_bf16(w1s)
        hi_t = w1hi.reshape(KT1, 128, HM, 128)
        lo_t = w1lo.reshape(KT1, 128, HM, 128)
        w1pk = np.empty((HM, 128, KT1, 2, 128), dtype=BF16)
        w1pk[:, :, :, 0, :] = hi_t.transpose(2, 1, 0, 3)
        w1pk[:, :, :, 1, :] = lo_t.transpose(2, 1, 0, 3)
        w1pn = np.ascontiguousarray(w1pk.reshape(HM, 128, KT1 * 256))

        # W2 even-position columns, packed in mm2 consumption order
        W2e = W2[:, c * CW:(c + 1) * CW].reshape(H, PS, 2 * V)[:, 0::2]
        W2e = W2e.reshape(H, CE)
        w2hi, w2lo = _split_bf16(W2e)
        w2pn = np.empty((128, NJ * CW), dtype=BF16)
        for g in range(HM):
            for s in range(NCORES):
                idx = g * NCORES + s
                r0 = s * HS + g * 128
                blk = np.empty((128, 4, CC), dtype=BF16)
                blk[:, 0] = w2hi[r0:r0 + 128, 0:CC]
                blk[:, 1] = w2lo[r0:r0 + 128, 0:CC]
                blk[:, 2] = w2hi[r0:r0 + 128, CC:2 * CC]
                blk[:, 3] = w2lo[r0:r0 + 128, CC:2 * CC]
                w2pn[:, idx * CW:(idx + 1) * CW] = blk.reshape(128, CW)

        b1pn = np.ascontiguousarray(
            b1[c * HS:(c + 1) * HS].reshape(HM, 128).T)
        b2s = b2[c * CW:(c + 1) * CW].reshape(PS, 2 * V)[0::2].reshape(CE)
        cols = slice(32 * c, 32 * c + 32, 2)
        inpe = inputs[:, cols, :].reshape(MT, 128, EP * V)
        inpp = np.ascontiguousarray(
            inpe.transpose(1, 0, 2).reshape(128, MT * EP * V))
        in_maps.append({
            "xtp": xtp,
            "w1p": w1pn,
            "w2p": np.ascontiguousarray(w2pn),
            "b1p": b1pn,
            "b2r": np.ascontiguousarray(np.broadcast_to(b2s, (128, CE))),
            "inpp": inpp,
        })

    res = run_bass_kernel_spmd(nc, in_maps, core_ids=list(range(NCORES)))
    _cache["last_result"] = res

    out = np.empty((B, L, V), np.float32)
    out[:, 1::2, :] = masked[:, 1::2, :]
    for c in range(NCORES):
        oe = res.results[c]["oute"].reshape(MT, 128, EP, V)
        out[:, 32 * c:32 * c + 32:2, :] = oe.reshape(B, EP, V)
    return out
